# revision 26
# baseline (speedup 1.0000x reference)
"""GNN attention (GAT-style single-target-node) kernel for 8 Trainium2 cores.

Problem:  x [32, 50000, 64], a [128, 1], node_index scalar, adj_mask [50000]
  tgt_score = x[:, idx] @ a[:64]                             # [B]
  e = leaky_relu(tgt_score[:, None] + x @ a[64:], 0.01)      # [B, N]
  attention = softmax(where(adj>0, e, -9e15), axis=1) * adj  # [B, N]

Sharding: data-parallel over batch (32 = 8 cores x 4 batches/core). Each
core computes complete softmax rows, so no cross-core reductions.

Per-core layout: nodes tile as [128 partitions, TK nodes, 64 feats];
scores accumulate into a [128, 400] grid (TF full tiles of TK plus a
[53, 16] tail).  Dot products: elementwise multiply against a replicated
a_src then a grouped free-axis reduce.  The multiply is split between
GPSIMD (k < GK) and DVE (k >= GK) to balance engine load, since DVE also
owns the reduces.  Softmax cross-partition max/sum go through a PE
transpose + ones-matmul broadcast (PE is otherwise idle).
"""

import numpy as np
from contextlib import ExitStack

import concourse.bass as bass
import concourse.bacc as bacc
import concourse.tile as tile
from concourse import mybir
from concourse.bass_utils import run_bass_kernel_spmd

B, N, D = 32, 50000, 64
NCORES = 8
BPC = B // NCORES            # batches per core
TK = 64                      # nodes per partition per full tile
TF = 6                       # full tiles: 6 * 128 * 64 = 49152 nodes
NFULL = TF * 128 * TK        # 49152
KT = 16                      # tail: nodes per partition
PT = (N - NFULL) // KT       # 53 partitions in tail tile
CF = TF * TK                 # 384 full-score columns
COLS = CF + KT               # 400 score columns
GK = 44                      # k < GK multiplies on GPSIMD, rest on DVE
GKT = 11                     # same split for the tail tile
REDUCE_MODE = "tree"         # "grouped": one reduce_sum; "tree": log2(D) adds
NEG = -9.0e15

F32 = mybir.dt.float32
AX = mybir.AxisListType
OP = mybir.AluOpType
ACT = mybir.ActivationFunctionType

TRACE = False                # set True (e.g. from test.py) to neuron-profile
LAST_RUN = None              # BassKernelResults of the most recent run

_CACHE = {}


def _build(reps=1):
    nc = bacc.Bacc(trn_type="TRN2", enable_partition_id=False,
                   num_devices=NCORES)
    xs = nc.dram_tensor("xs", [BPC, N, D], F32, kind="ExternalInput").ap()
    tgt_d = nc.dram_tensor("tgtvec", [128, BPC], F32, kind="ExternalInput").ap()
    arep_d = nc.dram_tensor("arep", [128, TK * D], F32, kind="ExternalInput").ap()
    mb_d = nc.dram_tensor("mbgrid", [128, COLS], F32, kind="ExternalInput").ap()
    id_d = nc.dram_tensor("ident", [128, 128], F32, kind="ExternalInput").ap()
    on_d = nc.dram_tensor("onesr", [1, 128], F32, kind="ExternalInput").ap()
    attn = nc.dram_tensor("attn", [BPC, N], F32, kind="ExternalOutput").ap()

    with tile.TileContext(nc) as tc, ExitStack() as ctx:
        singles = ctx.enter_context(tc.tile_pool(name="singles", bufs=1))
        xpool = ctx.enter_context(tc.tile_pool(name="xpool", bufs=4))
        ppool = ctx.enter_context(tc.tile_pool(name="ppool", bufs=3))
        spool = ctx.enter_context(tc.tile_pool(name="spool", bufs=2))
        epool = ctx.enter_context(tc.tile_pool(name="epool", bufs=2))
        stat = ctx.enter_context(tc.tile_pool(name="stat", bufs=8))
        pspool = ctx.enter_context(tc.tile_pool(name="ps", bufs=4, space="PSUM"))

        arep_sb = singles.tile([128, TK * D], F32)
        nc.sync.dma_start(out=arep_sb, in_=arep_d)
        arep3 = arep_sb[:].rearrange("p (k d) -> p k d", d=D)
        mb_sb = singles.tile([128, COLS], F32)
        nc.sync.dma_start(out=mb_sb, in_=mb_d)
        tgt_sb = singles.tile([128, BPC], F32)
        nc.sync.dma_start(out=tgt_sb, in_=tgt_d)
        ident = singles.tile([128, 128], F32)
        nc.sync.dma_start(out=ident, in_=id_d)
        onesr = singles.tile([1, 128], F32)
        nc.sync.dma_start(out=onesr, in_=on_d)

        def cross_partition(vec, op):
            """[128,1] per-partition stats -> [1,1] global (PE transpose)."""
            tp = pspool.tile([1, 128], F32, tag="ps")
            nc.tensor.transpose(tp, vec, ident)
            ct = stat.tile([1, 128], F32, tag="ct")
            nc.vector.tensor_copy(ct, tp)
            g1 = stat.tile([1, 1], F32, tag="g1")
            nc.vector.tensor_reduce(g1, ct, axis=AX.X, op=op)
            return g1

        def bcast_partitions(s1, tag):
            """[1,1] scalar -> [128,1] replicated (ones-matmul)."""
            bp = pspool.tile([128, 1], F32, tag="ps")
            nc.tensor.matmul(bp, onesr, s1, start=True, stop=True)
            out = stat.tile([128, 1], F32, tag=tag)
            nc.vector.tensor_copy(out, bp)
            return out

        def mul_split(pr, xt, ar, gk, k):
            if gk > 0:
                nc.gpsimd.tensor_mul(pr[:, :gk, :], xt[:, :gk, :], ar[:, :gk, :])
            if gk < k:
                nc.vector.tensor_mul(pr[:, gk:, :], xt[:, gk:, :], ar[:, gk:, :])

        def dot_reduce(sbcols, pr):
            """Sum pr [128, k, 64] over the last axis into sbcols [128, k]."""
            if REDUCE_MODE == "grouped":
                nc.vector.reduce_sum(sbcols, pr, axis=AX.X)
                return
            w = D // 2
            while w > 1:
                nc.vector.tensor_add(pr[:, :, 0:w], pr[:, :, 0:w],
                                     pr[:, :, w:2 * w])
                w //= 2
            nc.vector.tensor_add(sbcols, pr[:, :, 0], pr[:, :, 1])

        for _ in range(reps):
            for b in range(BPC):
                sb = spool.tile([128, COLS], F32)
                # tail-tile slots with no node behind them: keep them finite so
                # the masked add (-9e15) sends them to zero probability.  (The
                # tail reduce overwrites partitions < PT afterwards.)
                nc.vector.memset(sb[:, CF:], 0.0)
                for t in range(TF):
                    xt = xpool.tile([128, TK, D], F32)
                    nc.sync.dma_start(
                        out=xt,
                        in_=xs[b, t * 128 * TK:(t + 1) * 128 * TK, :]
                            .rearrange("(p k) d -> p k d", p=128),
                    )
                    pr = ppool.tile([128, TK, D], F32)
                    mul_split(pr, xt, arep3, GK, TK)
                    dot_reduce(sb[:, t * TK:(t + 1) * TK], pr)
                # tail tile: 848 nodes = [53 partitions, 16 nodes, 64 feats]
                xt_t = xpool.tile([128, KT, D], F32)
                nc.sync.dma_start(
                    out=xt_t[:PT],
                    in_=xs[b, NFULL:N, :].rearrange("(p k) d -> p k d", p=PT),
                )
                pr_t = ppool.tile([128, KT, D], F32)
                mul_split(pr_t[:PT], xt_t[:PT], arep3[:PT, :KT, :], GKT, KT)
                dot_reduce(sb[:PT, CF:], pr_t[:PT])

                # z = leaky_relu(scores + tgt, 0.01) + mask_bias.  Scores are
                # O(10), so exp(z) cannot overflow fp32 and the usual
                # max-subtraction is unnecessary (softmax is shift-invariant);
                # skipping it removes a whole PE/DVE round-trip per batch.
                z = epool.tile([128, COLS], F32)
                nc.vector.tensor_scalar_add(z, sb, tgt_sb[:, b:b + 1])
                nc.vector.scalar_tensor_tensor(z, z, 0.01, z,
                                               op0=OP.mult, op1=OP.max)
                nc.vector.tensor_add(z, z, mb_sb)

                pb = epool.tile([128, COLS], F32)
                srow = stat.tile([128, 1], F32)
                nc.scalar.activation(pb, z, ACT.Exp, bias=0.0, scale=1.0,
                                     accum_out=srow)
                gsum1 = cross_partition(srow, OP.add)
                rec1 = stat.tile([1, 1], F32)
                nc.vector.reciprocal(rec1, gsum1)
                rec = bcast_partitions(rec1, "rec")
                nc.vector.tensor_scalar_mul(pb, pb, rec)

                nc.sync.dma_start(
                    out=attn[b, 0:NFULL].rearrange("(t p k) -> p t k",
                                                   p=128, k=TK),
                    in_=pb[:, 0:CF].rearrange("p (t k) -> p t k", t=TF),
                )
                nc.sync.dma_start(
                    out=attn[b, NFULL:N].rearrange("(p k) -> p k", k=KT),
                    in_=pb[:PT, CF:],
                )
    nc.compile()
    return nc


def _host_prep(x, a, node_index, adj_mask):
    x = np.asarray(x, dtype=np.float32)
    a = np.asarray(a, dtype=np.float32).reshape(2 * D)
    adj = np.asarray(adj_mask).astype(np.int64)
    idx = int(node_index)
    a_tgt, a_src = a[:D], a[D:]

    tgt = (x[:, idx, :] @ a_tgt).astype(np.float32)          # [B]
    arep = np.tile(a_src, (128, TK)).astype(np.float32)      # [128, TK*D]

    mb = np.full((128, COLS), NEG, np.float32)
    m_full = adj[:NFULL].reshape(TF, 128, TK)
    mb[:, :CF] = np.where(
        m_full.transpose(1, 0, 2).reshape(128, CF) > 0, 0.0, NEG)
    m_tail = adj[NFULL:].reshape(PT, KT)
    mb[:PT, CF:] = np.where(m_tail > 0, 0.0, NEG)
    ident = np.eye(128, dtype=np.float32)
    onesr = np.ones((1, 128), dtype=np.float32)
    return x, tgt, arep, mb, ident, onesr


def _in_maps(x, tgt, arep, mb, ident, onesr):
    maps = []
    for c in range(NCORES):
        tv = np.tile(tgt[c * BPC:(c + 1) * BPC][None, :],
                     (128, 1)).astype(np.float32)
        maps.append({
            "xs": np.ascontiguousarray(x[c * BPC:(c + 1) * BPC]),
            "tgtvec": tv,
            "arep": arep,
            "mbgrid": mb,
            "ident": ident,
            "onesr": onesr,
        })
    return maps


def kernel(x, a, node_index, adj_mask):
    global LAST_RUN
    prep = _host_prep(x, a, node_index, adj_mask)
    if "nc" not in _CACHE:
        _CACHE["nc"] = _build()
    nc = _CACHE["nc"]
    res = run_bass_kernel_spmd(nc, _in_maps(*prep),
                               list(range(NCORES)), trace=TRACE)
    LAST_RUN = res
    return np.concatenate([res.results[c]["attn"] for c in range(NCORES)],
                          axis=0)


# revision 29
# speedup vs baseline: 2.1679x; 2.1679x over previous
"""GNN attention (GAT-style single-target-node) kernel for 8 Trainium2 cores.

Problem:  x [32, 50000, 64], a [128, 1], node_index scalar, adj_mask [50000]
  tgt_score = x[:, idx] @ a[:64]                             # [B]
  e = leaky_relu(tgt_score[:, None] + x @ a[64:], 0.01)      # [B, N]
  attention = softmax(where(adj>0, e, -9e15), axis=1) * adj  # [B, N]

Sharding: data-parallel over batch (32 = 8 cores x 4 batches/core). Each
core computes complete softmax rows, so no cross-core reductions.

Per-core layout: nodes tile as [128 partitions, TK nodes, 64 feats];
scores accumulate into a [128, 400] grid (TF full tiles of TK plus a
[53, 16] tail).  Dot products: elementwise multiply against a replicated
a_src then a grouped free-axis reduce.  The multiply is split between
GPSIMD (k < GK) and DVE (k >= GK) to balance engine load, since DVE also
owns the reduces.  Softmax cross-partition max/sum go through a PE
transpose + ones-matmul broadcast (PE is otherwise idle).
"""

import numpy as np
from contextlib import ExitStack

import jax
from jax.sharding import Mesh, PartitionSpec
from jax.experimental.shard_map import shard_map

import concourse.bass as bass
import concourse.bacc as bacc
import concourse.tile as tile
from concourse import mybir
from concourse.bass2jax import _bass_exec_p, install_neuronx_cc_hook

B, N, D = 32, 50000, 64
NCORES = 8
BPC = B // NCORES            # batches per core
TK = 64                      # nodes per partition per full tile
TF = 6                       # full tiles: 6 * 128 * 64 = 49152 nodes
NFULL = TF * 128 * TK        # 49152
KT = 16                      # tail: nodes per partition
PT = (N - NFULL) // KT       # 53 partitions in tail tile
CF = TF * TK                 # 384 full-score columns
COLS = CF + KT               # 400 score columns
GK = 44                      # k < GK multiplies on GPSIMD, rest on DVE
GKT = 11                     # same split for the tail tile
REDUCE_MODE = "tree"         # "grouped": one reduce_sum; "tree": log2(D) adds
NEG = -9.0e15

F32 = mybir.dt.float32
AX = mybir.AxisListType
OP = mybir.AluOpType
ACT = mybir.ActivationFunctionType

TRACE = False                # set True (e.g. from test.py) to neuron-profile
LAST_RUN = None              # BassKernelResults of the most recent run

_CACHE = {}


def _build(reps=1):
    nc = bacc.Bacc(trn_type="TRN2", enable_partition_id=False,
                   num_devices=NCORES)
    xs = nc.dram_tensor("xs", [BPC, N, D], F32, kind="ExternalInput").ap()
    tgt_d = nc.dram_tensor("tgtvec", [128, BPC], F32, kind="ExternalInput").ap()
    arep_d = nc.dram_tensor("arep", [128, TK * D], F32, kind="ExternalInput").ap()
    mb_d = nc.dram_tensor("mbgrid", [128, COLS], F32, kind="ExternalInput").ap()
    id_d = nc.dram_tensor("ident", [128, 128], F32, kind="ExternalInput").ap()
    on_d = nc.dram_tensor("onesr", [1, 128], F32, kind="ExternalInput").ap()
    attn = nc.dram_tensor("attn", [BPC, N], F32, kind="ExternalOutput").ap()

    with tile.TileContext(nc) as tc, ExitStack() as ctx:
        singles = ctx.enter_context(tc.tile_pool(name="singles", bufs=1))
        xpool = ctx.enter_context(tc.tile_pool(name="xpool", bufs=4))
        ppool = ctx.enter_context(tc.tile_pool(name="ppool", bufs=3))
        spool = ctx.enter_context(tc.tile_pool(name="spool", bufs=2))
        epool = ctx.enter_context(tc.tile_pool(name="epool", bufs=2))
        stat = ctx.enter_context(tc.tile_pool(name="stat", bufs=8))
        pspool = ctx.enter_context(tc.tile_pool(name="ps", bufs=4, space="PSUM"))

        arep_sb = singles.tile([128, TK * D], F32)
        nc.sync.dma_start(out=arep_sb, in_=arep_d)
        arep3 = arep_sb[:].rearrange("p (k d) -> p k d", d=D)
        mb_sb = singles.tile([128, COLS], F32)
        nc.sync.dma_start(out=mb_sb, in_=mb_d)
        tgt_sb = singles.tile([128, BPC], F32)
        nc.sync.dma_start(out=tgt_sb, in_=tgt_d)
        ident = singles.tile([128, 128], F32)
        nc.sync.dma_start(out=ident, in_=id_d)
        onesr = singles.tile([1, 128], F32)
        nc.sync.dma_start(out=onesr, in_=on_d)

        def cross_partition(vec, op):
            """[128,1] per-partition stats -> [1,1] global (PE transpose)."""
            tp = pspool.tile([1, 128], F32, tag="ps")
            nc.tensor.transpose(tp, vec, ident)
            ct = stat.tile([1, 128], F32, tag="ct")
            nc.vector.tensor_copy(ct, tp)
            g1 = stat.tile([1, 1], F32, tag="g1")
            nc.vector.tensor_reduce(g1, ct, axis=AX.X, op=op)
            return g1

        def bcast_partitions(s1, tag):
            """[1,1] scalar -> [128,1] replicated (ones-matmul)."""
            bp = pspool.tile([128, 1], F32, tag="ps")
            nc.tensor.matmul(bp, onesr, s1, start=True, stop=True)
            out = stat.tile([128, 1], F32, tag=tag)
            nc.vector.tensor_copy(out, bp)
            return out

        def mul_split(pr, xt, ar, gk, k):
            if gk > 0:
                nc.gpsimd.tensor_mul(pr[:, :gk, :], xt[:, :gk, :], ar[:, :gk, :])
            if gk < k:
                nc.vector.tensor_mul(pr[:, gk:, :], xt[:, gk:, :], ar[:, gk:, :])

        def dot_reduce(sbcols, pr):
            """Sum pr [128, k, 64] over the last axis into sbcols [128, k]."""
            if REDUCE_MODE == "grouped":
                nc.vector.reduce_sum(sbcols, pr, axis=AX.X)
                return
            w = D // 2
            while w > 1:
                nc.vector.tensor_add(pr[:, :, 0:w], pr[:, :, 0:w],
                                     pr[:, :, w:2 * w])
                w //= 2
            nc.vector.tensor_add(sbcols, pr[:, :, 0], pr[:, :, 1])

        for _ in range(reps):
            for b in range(BPC):
                sb = spool.tile([128, COLS], F32)
                # tail-tile slots with no node behind them: keep them finite so
                # the masked add (-9e15) sends them to zero probability.  (The
                # tail reduce overwrites partitions < PT afterwards.)
                nc.vector.memset(sb[:, CF:], 0.0)
                for t in range(TF):
                    xt = xpool.tile([128, TK, D], F32)
                    nc.sync.dma_start(
                        out=xt,
                        in_=xs[b, t * 128 * TK:(t + 1) * 128 * TK, :]
                            .rearrange("(p k) d -> p k d", p=128),
                    )
                    pr = ppool.tile([128, TK, D], F32)
                    mul_split(pr, xt, arep3, GK, TK)
                    dot_reduce(sb[:, t * TK:(t + 1) * TK], pr)
                # tail tile: 848 nodes = [53 partitions, 16 nodes, 64 feats]
                xt_t = xpool.tile([128, KT, D], F32)
                nc.sync.dma_start(
                    out=xt_t[:PT],
                    in_=xs[b, NFULL:N, :].rearrange("(p k) d -> p k d", p=PT),
                )
                pr_t = ppool.tile([128, KT, D], F32)
                mul_split(pr_t[:PT], xt_t[:PT], arep3[:PT, :KT, :], GKT, KT)
                dot_reduce(sb[:PT, CF:], pr_t[:PT])

                # z = leaky_relu(scores + tgt, 0.01) + mask_bias.  Scores are
                # O(10), so exp(z) cannot overflow fp32 and the usual
                # max-subtraction is unnecessary (softmax is shift-invariant);
                # skipping it removes a whole PE/DVE round-trip per batch.
                z = epool.tile([128, COLS], F32)
                nc.vector.tensor_scalar_add(z, sb, tgt_sb[:, b:b + 1])
                nc.vector.scalar_tensor_tensor(z, z, 0.01, z,
                                               op0=OP.mult, op1=OP.max)
                nc.vector.tensor_add(z, z, mb_sb)

                pb = epool.tile([128, COLS], F32)
                srow = stat.tile([128, 1], F32)
                nc.scalar.activation(pb, z, ACT.Exp, bias=0.0, scale=1.0,
                                     accum_out=srow)
                gsum1 = cross_partition(srow, OP.add)
                rec1 = stat.tile([1, 1], F32)
                nc.vector.reciprocal(rec1, gsum1)
                rec = bcast_partitions(rec1, "rec")
                nc.vector.tensor_scalar_mul(pb, pb, rec)

                nc.sync.dma_start(
                    out=attn[b, 0:NFULL].rearrange("(t p k) -> p t k",
                                                   p=128, k=TK),
                    in_=pb[:, 0:CF].rearrange("p (t k) -> p t k", t=TF),
                )
                nc.sync.dma_start(
                    out=attn[b, NFULL:N].rearrange("(p k) -> p k", k=KT),
                    in_=pb[:PT, CF:],
                )
    nc.compile()
    return nc


def _host_prep(x, a, node_index, adj_mask):
    x = np.asarray(x, dtype=np.float32)
    a = np.asarray(a, dtype=np.float32).reshape(2 * D)
    adj = np.asarray(adj_mask).astype(np.int64)
    idx = int(node_index)
    a_tgt, a_src = a[:D], a[D:]

    tgt = (x[:, idx, :] @ a_tgt).astype(np.float32)          # [B]
    arep = np.tile(a_src, (128, TK)).astype(np.float32)      # [128, TK*D]

    mb = np.full((128, COLS), NEG, np.float32)
    m_full = adj[:NFULL].reshape(TF, 128, TK)
    mb[:, :CF] = np.where(
        m_full.transpose(1, 0, 2).reshape(128, CF) > 0, 0.0, NEG)
    m_tail = adj[NFULL:].reshape(PT, KT)
    mb[:PT, CF:] = np.where(m_tail > 0, 0.0, NEG)
    ident = np.eye(128, dtype=np.float32)
    onesr = np.ones((1, 128), dtype=np.float32)
    return x, tgt, arep, mb, ident, onesr


def _in_maps(x, tgt, arep, mb, ident, onesr):
    maps = []
    for c in range(NCORES):
        tv = np.tile(tgt[c * BPC:(c + 1) * BPC][None, :],
                     (128, 1)).astype(np.float32)
        maps.append({
            "xs": np.ascontiguousarray(x[c * BPC:(c + 1) * BPC]),
            "tgtvec": tv,
            "arep": arep,
            "mbgrid": mb,
            "ident": ident,
            "onesr": onesr,
        })
    return maps


def _runner():
    """Build the Bass program once and wrap its NEFF custom call in a jitted
    shard_map over the 8 cores.  Cached so repeat kernel() calls only pay
    input upload + execution, not re-tracing/compiling."""
    if "runner" in _CACHE:
        return _CACHE["runner"]
    install_neuronx_cc_hook()
    nc = _CACHE.setdefault("nc", _build())
    in_names, out_names, out_avals, zero_shapes = [], [], [], []
    for alloc in nc.m.functions[0].allocations:
        if not isinstance(alloc, mybir.MemoryLocationSet):
            continue
        name = alloc.memorylocations[0].name
        if alloc.kind == "ExternalInput":
            in_names.append(name)
        elif alloc.kind == "ExternalOutput":
            out_names.append(name)
            shape = tuple(alloc.tensor_shape)
            dtype = mybir.dt.np(alloc.dtype)
            out_avals.append(jax.core.ShapedArray(shape, dtype))
            zero_shapes.append((shape, dtype))

    def _body(*args):
        return tuple(_bass_exec_p.bind(
            *args,
            out_avals=tuple(out_avals),
            in_names=tuple(in_names + out_names),
            out_names=tuple(out_names),
            lowering_input_output_aliases=(),
            sim_require_finite=True,
            sim_require_nnan=True,
            nc=nc,
        ))

    mesh = Mesh(np.asarray(jax.devices()[:NCORES]), ("core",))
    nin = len(in_names) + len(out_names)
    sharded = jax.jit(shard_map(
        _body, mesh=mesh,
        in_specs=(PartitionSpec("core"),) * nin,
        out_specs=(PartitionSpec("core"),) * len(out_names),
        check_rep=False))
    _CACHE["runner"] = (sharded, in_names, out_names, zero_shapes)
    return _CACHE["runner"]


def kernel(x, a, node_index, adj_mask):
    global LAST_RUN
    prep = _host_prep(x, a, node_index, adj_mask)
    maps = _in_maps(*prep)
    sharded, in_names, out_names, zero_shapes = _runner()
    # concat of the 8 per-core xs shards is exactly the full x — skip the copy
    ins = [prep[0] if nm == "xs" else
           np.concatenate([m[nm] for m in maps], axis=0) for nm in in_names]
    zeros = [np.zeros((NCORES * s[0], *s[1:]), d) for s, d in zero_shapes]
    outs = sharded(*ins, *zeros)
    LAST_RUN = outs
    attn = np.asarray(outs[out_names.index("attn")])  # [NCORES*BPC, N]
    return attn.reshape(B, N)
